# revision 1
# baseline (speedup 1.0000x reference)
"""MoE domain-gate routing kernel for Trainium2 (8 NeuronCores, token-sharded).

Computes, for T=8192 tokens, E=16 experts, capacity C=512:
  combine[t, e, c] = 1.0 iff token t routed to expert e at capacity slot c
  dispatch = combine != 0
  l_aux = 0.0

Per-core plan (1024 tokens each):
  * Host merges (domain_ids, mask) into a single f32 "mids" stream
    (padded tokens -> sentinel 100), laid out [128 partitions, 64 blocks]
    with the 64 token-blocks ROTATED so each core's own 8 blocks come
    first; a per-core 0/1 prefix matrix W encodes which (rotated) blocks
    precede each local block globally. This keeps the device program
    identical across cores (SPMD) with all per-core variation in data.
  * Device: one-hot X[p,(b,e)] = (mids==e); block sums via PE column-sum
    matmul; exclusive per-(local block, expert) offsets via matmul with W;
    within-block inclusive cumsum via matmul with upper-triangular ones;
    offsets broadcast-added via a K=1 matmul into the same PSUM tile.
    From the resulting global positions: capacity filter, slot index,
    and a flat index idx = e*512 + slot (or a sentinel if dropped).
  * Output: for each 128-token row block, compare an iota row [0..8191]
    against idx per partition -> [128, 8192] f32 combine tile (+ u8 cast
    for dispatch), DMA to DRAM. 40MB of output per core; memory-bound.
"""

import numpy as np

import concourse.bacc as bacc
import concourse.mybir as mybir
import concourse.tile as tile
from concourse.bass_utils import run_bass_kernel_spmd

_T, _E, _C = 8192, 16, 512
_NC = 8  # cores
_B = 64  # global 128-token blocks
_BL = 8  # local blocks per core
_TL = _B // _NC * 128  # tokens per core = 1024
_W = _E * _C  # flat output row width = 8192
_SENT = 1.0e6  # flat-index sentinel for dropped/padded tokens

_f32 = mybir.dt.float32
_u8 = mybir.dt.uint8

_nc_cache = []


def _build_nc():
    nc = bacc.Bacc("TRN2", target_bir_lowering=False, debug=False, num_devices=_NC)

    mids_d = nc.dram_tensor("mids", [128, _B], _f32, kind="ExternalInput")
    w_d = nc.dram_tensor("w", [_B, _BL], _f32, kind="ExternalInput")
    u_d = nc.dram_tensor("u", [128, 128], _f32, kind="ExternalInput")
    comb_d = nc.dram_tensor("comb", [_TL, _W], _f32, kind="ExternalOutput")
    disp_d = nc.dram_tensor("disp", [_TL, _W], _u8, kind="ExternalOutput")

    with tile.TileContext(nc) as tc:
        with (
            tc.tile_pool(name="consts", bufs=1) as cpool,
            tc.tile_pool(name="work", bufs=1) as wpool,
            tc.tile_pool(name="bigc", bufs=3) as bigc,
            tc.tile_pool(name="bigd", bufs=3) as bigd,
            tc.tile_pool(name="psum", bufs=1, space="PSUM") as ppool,
        ):
            mids_t = cpool.tile([128, _B], _f32)
            w_t = cpool.tile([_B, _BL], _f32)
            u_t = cpool.tile([128, 128], _f32)
            iota_t = cpool.tile([128, _W], _f32)
            nc.sync.dma_start(mids_t[:], mids_d[:])
            nc.sync.dma_start(w_t[:], w_d[:])
            nc.sync.dma_start(u_t[:], u_d[:])
            nc.gpsimd.iota(
                iota_t[:],
                pattern=[[1, _W]],
                base=0,
                channel_multiplier=0,
                allow_small_or_imprecise_dtypes=True,
            )

            # one-hot X[p, b*16+e] = (mids[p, b] == e), f32
            x_t = wpool.tile([128, _B * _E], _f32)
            x3 = x_t[:].rearrange("p (b e) -> p b e", e=_E)
            for e in range(_E):
                nc.vector.tensor_single_scalar(
                    x3[:, :, e], mids_t[:], float(e), mybir.AluOpType.is_equal
                )

            # per-(block, expert) counts: column sums of X via matmul with ones
            row_p = ppool.tile([1, _B * _E], _f32)
            nc.tensor.matmul(
                row_p[:, 0:512], u_t[:, 127:128], x_t[:, 0:512], start=True, stop=True
            )
            nc.tensor.matmul(
                row_p[:, 512:1024], u_t[:, 127:128], x_t[:, 512:1024], start=True, stop=True
            )
            row_s = wpool.tile([1, _B * _E], _f32)
            nc.vector.tensor_copy(row_s[:], row_p[:])
            # reshape [1, 1024] -> [64, 16] (blocks onto partitions)
            bsums_t = wpool.tile([_B, _E], _f32)
            nc.sync.dma_start(bsums_t[:], row_s[:])

            # exclusive offsets per (local block, expert): W^T @ bsums -> [8, 16]
            o_p = ppool.tile([_BL, _E], _f32)
            nc.tensor.matmul(o_p[:], w_t[:], bsums_t[:], start=True, stop=True)
            o_s = wpool.tile([_BL, _E], _f32)
            nc.vector.tensor_copy(o_s[:], o_p[:])
            # reshape [8, 16] -> [1, 128]
            orow_t = wpool.tile([1, _BL * _E], _f32)
            nc.sync.dma_start(orow_t[:], o_s[:])

            # pos[p, j*16+e] = within-block inclusive cumsum + offset (global
            # inclusive count for this token's expert)
            p1 = ppool.tile([128, 128], _f32)
            nc.tensor.matmul(p1[:], u_t[:], x_t[:, 0:128], start=True, stop=False)
            nc.tensor.matmul(p1[:], u_t[0:1, :], orow_t[:], start=False, stop=True)

            # capacity filter and slot index
            sel = wpool.tile([128, 128], _f32)
            nc.vector.tensor_single_scalar(
                sel[:], p1[:], float(_C), mybir.AluOpType.is_le
            )
            g_t = wpool.tile([128, 128], _f32)
            nc.vector.tensor_mul(g_t[:], sel[:], x_t[:, 0:128])
            pos1 = wpool.tile([128, 128], _f32)
            nc.vector.tensor_scalar_add(pos1[:], p1[:], -1.0)
            locg = wpool.tile([128, 128], _f32)
            nc.vector.tensor_mul(locg[:], pos1[:], g_t[:])
            loc_s = wpool.tile([128, _BL], _f32)
            nc.vector.reduce_sum(
                loc_s[:],
                locg[:].rearrange("p (j e) -> p j e", e=_E),
                axis=mybir.AxisListType.X,
            )
            kept = wpool.tile([128, _BL], _f32)
            nc.vector.reduce_sum(
                kept[:],
                g_t[:].rearrange("p (j e) -> p j e", e=_E),
                axis=mybir.AxisListType.X,
            )
            # idx = (e*512 + slot) if kept else SENT
            t1 = wpool.tile([128, _BL], _f32)
            nc.vector.tensor_single_scalar(
                t1[:], mids_t[:, 0:_BL], float(_C), mybir.AluOpType.mult
            )
            t2 = wpool.tile([128, _BL], _f32)
            nc.vector.tensor_add(t2[:], t1[:], loc_s[:])
            t3 = wpool.tile([128, _BL], _f32)
            nc.vector.tensor_mul(t3[:], t2[:], kept[:])
            t4 = wpool.tile([128, _BL], _f32)
            nc.vector.tensor_scalar(
                t4[:], kept[:], -_SENT, _SENT, mybir.AluOpType.mult, mybir.AluOpType.add
            )
            idx_t = wpool.tile([128, _BL], _f32)
            nc.vector.tensor_add(idx_t[:], t3[:], t4[:])

            # output generation: one 128-token row block at a time
            for j in range(_BL):
                comb_t = bigc.tile([128, _W], _f32)
                nc.vector.tensor_single_scalar(
                    comb_t[:], iota_t[:], idx_t[:, j : j + 1], mybir.AluOpType.is_equal
                )
                disp_t = bigd.tile([128, _W], _u8)
                nc.scalar.activation(
                    disp_t[:], comb_t[:], mybir.ActivationFunctionType.Copy
                )
                nc.sync.dma_start(comb_d[j * 128 : (j + 1) * 128, :], comb_t[:])
                nc.sync.dma_start(disp_d[j * 128 : (j + 1) * 128, :], disp_t[:])

    nc.compile()
    return nc


def _get_nc():
    if not _nc_cache:
        _nc_cache.append(_build_nc())
    return _nc_cache[0]


def _make_in_maps(domain_ids, mask):
    ids = np.asarray(domain_ids).reshape(_T).astype(np.int64)
    m = np.asarray(mask).reshape(_T).astype(bool)
    mids = np.where(m, 100, ids).astype(np.float32)
    # [p, b] layout: token b*128 + p
    mids_pb = np.ascontiguousarray(mids.reshape(_B, 128).T)
    u_np = np.triu(np.ones((128, 128), dtype=np.float32))
    in_maps = []
    for c in range(_NC):
        rot = np.ascontiguousarray(np.roll(mids_pb, -_BL * c, axis=1))
        g = (np.arange(_B) + _BL * c) % _B  # global block of rotated column r
        w_np = (g[:, None] < (_BL * c + np.arange(_BL))[None, :]).astype(np.float32)
        in_maps.append({"mids": rot, "w": np.ascontiguousarray(w_np), "u": u_np})
    return in_maps


def _assemble(results):
    comb = np.empty((_T, _E, _C), dtype=np.float32)
    disp = np.empty((_T, _E, _C), dtype=bool)
    for c in range(_NC):
        comb[c * _TL : (c + 1) * _TL] = results[c]["comb"].reshape(_TL, _E, _C)
        disp[c * _TL : (c + 1) * _TL] = (
            results[c]["disp"].view(np.bool_).reshape(_TL, _E, _C)
        )
    return comb, disp


def kernel(**inputs):
    nc = _get_nc()
    in_maps = _make_in_maps(inputs["domain_ids"], inputs["mask"])
    r = run_bass_kernel_spmd(nc, in_maps, core_ids=list(range(_NC)))
    comb, disp = _assemble(r.results)
    l_aux = np.zeros((), dtype=np.float32)
    return (l_aux, comb, disp)


def kernel_traced(inputs, trace_cores=None):
    """Dev helper: run with NTFF profiling, return (outputs, BassKernelResults)."""
    nc = _get_nc()
    in_maps = _make_in_maps(inputs["domain_ids"], inputs["mask"])
    r = run_bass_kernel_spmd(
        nc,
        in_maps,
        core_ids=list(range(_NC)),
        trace=True,
        trace_cores=trace_cores or [0],
    )
    comb, disp = _assemble(r.results)
    return (np.zeros((), dtype=np.float32), comb, disp), r
